# revision 32
# baseline (speedup 1.0000x reference)
"""KANLinear (no residual) Trainium2 kernel.

out[b,o] = sum_{i,g} B_g(x[b,i]) * W[o,i,g] where B_g are cubic B-spline
bases on a uniform grid (G=5, k=3, range [-1,1] -> 8 bases, knots
t_j = 0.4*j - 2.2).

Closed form used on-device: with u = 2.5*x + 5.5 - g and q = |u - 2|
(symmetry fold about the basis center),

    B_g(x) = relu((2-q)*c1)^3 - relu((1-q)*c2)^3
    c1 = 6^(-1/3), c2 = (2/3)^(1/3)

which is exact for the cardinal cubic B-spline everywhere. Two custom DVE
ops per basis plane using the ABSOLUTE_DIFF ALU stage:

    op1 (6 stages): h  = cube(relu(imm2 - |x*s0 - s1|))
    op2 (7 stages): B  = h - cube(relu(imm2 - |x*s0 - s1|))   (h via Src1)

so the h1-h2 combine is fused into the second op — no separate
tensor_sub pass. Bases are written fp16 and the big matmul runs fp16
(1 cycle/row on the PE, same as fp32r, half the SBUF/DMA traffic of
fp32). spline_weight is pre-converted to fp16 on host (rel err 2^-11,
far inside the 2e-2 gate).

The pipeline is (tile-pair, g)-granular: each DVE block produces a
[128, 1024] basis plane covering two 128-in-feature tiles; the 16
dependent matmuls (2 K-tiles x 4 M x 2 N) start as soon as it lands.
Per-block DVE time (~2.3us) < tensor time (~3.4us), so the PE array
stays fed and never drops out of its p-state.

Sharding: data-parallel over tokens (4096 -> 512 per core on 8 cores),
spline_weight replicated; no collectives, host concatenates the shards.
"""

import numpy as np

N_CORES = 8
B_TOT = 4096
B_SHARD = B_TOT // N_CORES  # 512
IN_F = 1024
OUT_F = 1024
G = 8  # GRID_SIZE + SPLINE_ORDER
I_TILES = IN_F // 128  # 8
M_TILES = B_SHARD // 128  # 4
N_CHUNKS = OUT_F // 512  # 2
N_WARMUP_FULL = 9  # N=512 PE warmup matmuls (bridge bulk of x DMA latency)
N_WARMUP_SHORT = 8  # N=128 trailing warmups (fine-grained handoff to real work)

_C1 = float(6.0 ** (-1.0 / 3.0))
_C2 = float((2.0 / 3.0) ** (1.0 / 3.0))

_CACHE = {}


def _hinge_ops():
    """Register (once) and return the two custom DVE ops:

    op1: out = cube(relu(imm2 - |in0*s0 - s1|))            (6 ALU stages)
    op2: out = in1 - cube(relu(imm2 - |in0*s0 - s1|))      (7 ALU stages)

    |.| is a single ABSOLUTE_DIFF stage; cube is sq+mult."""
    if "ops" in _CACHE:
        return _CACHE["ops"]

    from concourse import dve_ops
    from concourse.dve_ops import DveOp
    from concourse.dve_spec import (
        AluOp,
        Bin,
        C0,
        C1,
        C2,
        Spec,
        Src0,
        Src1,
        lower,
        relu,
        sq,
    )
    from concourse.dve_uop import DveOpSpec

    def absdiff(x, y):
        return Bin(AluOp.ABSOLUTE_DIFF, x, y)

    def _hinge_body():
        m = Src0 * C0
        a = absdiff(m, C1)
        z = C2 - a
        r = relu(z)
        return sq(r) * r

    def _ref1(in0, in1, s0, s1, imm2):
        t = np.abs(in0.astype(np.float32) * np.float32(s0) - np.float32(s1))
        r = np.maximum(np.float32(imm2) - t, np.float32(0.0))
        return (r * r * r).astype(np.float32)

    def _ref2(in0, in1, s0, s1, imm2):
        t = np.abs(in0.astype(np.float32) * np.float32(s0) - np.float32(s1))
        r = np.maximum(np.float32(imm2) - t, np.float32(0.0))
        return (in1.astype(np.float32) - r * r * r).astype(np.float32)

    spec1 = Spec(body=_hinge_body(), reference=_ref1)
    spec2 = Spec(body=Src1 - _hinge_body(), reference=_ref2)

    ops = []
    for name, spec in (
        ("BSPLINE_HINGE_CUBE6", spec1),
        ("BSPLINE_BASIS_FUSED", spec2),
    ):
        if name not in dve_ops._SUB_OPCODE_FOR_NAME:
            row = dve_ops._CUSTOM_DVE_ROW_BASE + len(dve_ops.OPS)
            assert row < 0x20
            shas = {}
            for ver in ("v3", "v4"):
                try:
                    tmp = DveOpSpec(
                        name=name,
                        opcode=row,
                        uops=lower(spec, ver=ver),
                        rd1_en=(name == "BSPLINE_BASIS_FUSED"),
                    )
                    shas[ver] = tmp.sha(ver)
                except Exception:
                    pass
            op = DveOp(name, spec, subdim=False, uops_sha=shas)
            dve_ops.OPS.append(op)
            dve_ops._SUB_OPCODE_FOR_NAME[name] = row
            dve_ops.CUSTOM_DVE_SPECS[name] = spec
        else:
            op = next(o for o in dve_ops.OPS if o.name == name)
        ops.append(op)

    _CACHE["ops"] = tuple(ops)
    return _CACHE["ops"]


def _build_nc():
    """Build the per-core Bass program (SPMD: identical on all 8 cores)."""
    if "nc" in _CACHE:
        return _CACHE["nc"]

    from concourse import bacc
    import concourse.mybir as mybir
    import concourse.tile as tile

    op1, op2 = _hinge_ops()

    f32 = mybir.dt.float32
    f16 = mybir.dt.float16

    nc = bacc.Bacc(None, target_bir_lowering=False)

    x_t = nc.declare_dram_parameter("x_t", [IN_F, B_SHARD], f32, isOutput=False)
    w = nc.declare_dram_parameter("w", [G * IN_F, OUT_F], f16, isOutput=False)
    # fp16 device output (half the drain DMA bytes); host converts to fp32.
    # |out| ~ 2, fp16 rounding adds ~3e-4 relative -- far inside the gate.
    out = nc.declare_dram_parameter("out", [B_SHARD, OUT_F], f16, isOutput=True)

    with tile.TileContext(nc) as tc:
        with (
            tc.tile_pool(name="xp", bufs=3) as xp,
            tc.tile_pool(name="hp", bufs=1) as hp,
            tc.tile_pool(name="bp", bufs=3) as bp,
            tc.tile_pool(name="wp", bufs=8) as wp,
            tc.tile_pool(name="wup", bufs=1) as wup,
            tc.tile_pool(name="outp", bufs=4) as outp,
            tc.tile_pool(name="ps", bufs=1, space="PSUM") as ps,
        ):
            # one [128,1024] PSUM tile per m spans two adjacent banks; each
            # matmul targets a single-bank [128,512] slice, and the drain
            # copies the pair in one FD=1024 pass.
            psum = [
                ps.tile([128, 1024], f32, tag=f"ps_{m}", name=f"ps_{m}")
                for m in range(M_TILES)
            ]

            # t=0 x tile is latency-critical: its two half-DMAs issue first
            # (sync + gpsimd queues in parallel), ahead of everything else.
            # Latency-critical startup loads, spread over the three DMA-capable
            # engines. The first 4 matmuls only need wt_pre0's n0 half, so it
            # is issued first and alone on scalar; x0's halves go on sync +
            # scalar; the rest of the first two weight tiles go on gpsimd.
            xt0 = xp.tile([128, B_SHARD], f32, tag="xt", name="xt0")
            wt_p0 = wp.tile([128, OUT_F], f16, tag="wt", name="wt_pre0")
            wt_p1 = wp.tile([128, OUT_F], f16, tag="wt", name="wt_pre1")
            nc.sync.dma_start(out=xt0[:, 0:256], in_=x_t[0:128, 0:256])
            nc.scalar.dma_start(out=wt_p0[:, 0:512], in_=w[0:128, 0:512])
            nc.scalar.dma_start(out=xt0[:, 256:512], in_=x_t[0:128, 256:512])
            nc.gpsimd.dma_start(out=wt_p0[:, 512:1024], in_=w[0:128, 512:1024])
            nc.gpsimd.dma_start(out=wt_p1[:, :], in_=w[IN_F : IN_F + 128, :])
            pre_wt = [wt_p0, wt_p1]

            # PE p-state warmup: zero matmuls into bank (0,0) keep the
            # tensor engine busy (and ramping to full clock) while the
            # first x DMA + basis ops are still in flight. The real
            # accumulation group's start=True reset discards them. The
            # trailing short (N=128) warmups bound the PE handoff idle to
            # ~150ns whenever the first real operands land.
            wu = wup.tile([128, 512], f16, tag="wu")
            nc.vector.memset(wu[:, :], 0)
            for k in range(N_WARMUP_FULL + N_WARMUP_SHORT):
                ncols = 512 if k < N_WARMUP_FULL else 128
                nc.tensor.matmul(
                    psum[0][:, :ncols],
                    wu[:, :128],
                    wu[:, :ncols],
                    start=True,
                    stop=True,
                    skip_group_check=True,
                )

            # x tiles are prefetched one-ahead from inside the previous
            # tile's g-loop (at g==4) so their DMA packets never sit in
            # front of the latency-critical first weight tiles.
            xts = [xt0] + [None] * (I_TILES - 1)
            for t in range(I_TILES):
                xt = xts[t]

                for g in range(G):
                    if g == 4 and t + 1 < I_TILES:
                        xn = xp.tile([128, B_SHARD], f32, tag="xt")
                        nc.sync.dma_start(
                            out=xn[:, :],
                            in_=x_t[(t + 1) * 128 : (t + 2) * 128, :],
                        )
                        xts[t + 1] = xn
                    h1 = hp.tile([128, B_SHARD], f32, tag="h1")
                    nc.vector._custom_dve(
                        op1,
                        out=h1[:, :],
                        in0=xt[:, :],
                        s0=2.5 * _C1,
                        s1=(g - 3.5) * _C1,
                        imm2=2.0 * _C1,
                    )
                    bg = bp.tile([128, B_SHARD], f16, tag="bg")
                    nc.vector._custom_dve(
                        op2,
                        out=bg[:, :],
                        in0=xt[:, :],
                        in1=h1[:, :],
                        s0=2.5 * _C2,
                        s1=(g - 3.5) * _C2,
                        imm2=1.0 * _C2,
                    )

                    if t == 0 and g < 2:
                        wt = pre_wt[g]  # DMA already issued up front
                    else:
                        wt = wp.tile([128, OUT_F], f16, tag="wt")
                        r0 = g * IN_F + t * 128
                        # issue wt DMAs from the two otherwise-idle engines
                        # so the sync queue only carries x/out DMAs
                        dma_eng = nc.scalar if (t * G + g) % 2 == 0 else nc.gpsimd
                        dma_eng.dma_start(out=wt[:, :], in_=w[r0 : r0 + 128, :])
                    first = t == 0 and g == 0
                    last = t == I_TILES - 1 and g == G - 1
                    for m in range(M_TILES):
                        lhsT = bg[:, m * 128 : (m + 1) * 128]
                        for n in range(N_CHUNKS):
                            nc.tensor.matmul(
                                psum[m][:, n * 512 : (n + 1) * 512],
                                lhsT,
                                wt[:, n * 512 : (n + 1) * 512],
                                start=first,
                                stop=last,
                            )

            for m in range(M_TILES):
                ot = outp.tile([128, OUT_F], f16, tag="ot")
                if m % 2 == 0:
                    nc.scalar.copy(out=ot[:, :], in_=psum[m][:, :])
                else:
                    nc.vector.tensor_copy(out=ot[:, :], in_=psum[m][:, :])
                nc.sync.dma_start(out=out[m * 128 : (m + 1) * 128, :], in_=ot[:, :])

    nc.finalize()
    _CACHE["nc"] = nc
    return nc


def _in_maps(x, w2):
    maps = []
    for c in range(N_CORES):
        xs = x[c * B_SHARD : (c + 1) * B_SHARD, :]
        maps.append({"x_t": np.ascontiguousarray(xs.T), "w": w2})
    return maps


def kernel(x, spline_weight, _trace=False):
    x = np.ascontiguousarray(np.asarray(x, dtype=np.float32))
    W = np.asarray(spline_weight, dtype=np.float32)
    assert x.shape == (B_TOT, IN_F) and W.shape == (OUT_F, IN_F, G)

    # w2[g*IN_F + i, o] = W[o, i, g], fp16
    w2 = np.ascontiguousarray(
        W.transpose(2, 1, 0).reshape(G * IN_F, OUT_F).astype(np.float16)
    )

    from concourse.bass_utils import run_bass_kernel_spmd

    nc = _build_nc()
    res = run_bass_kernel_spmd(nc, _in_maps(x, w2), list(range(N_CORES)), trace=_trace)
    out = np.concatenate(
        [np.asarray(res.results[c]["out"]).astype(np.float32) for c in range(N_CORES)],
        axis=0,
    )
    if _trace:
        _CACHE["last_result"] = res
    return out
